# revision 11
# baseline (speedup 1.0000x reference)
"""Distributed GQA attention kernel for 8 Trainium2 NeuronCores.

Problem: B=1, S=2048, DIM=2048, 32 Q heads / 8 KV heads, head_dim 64,
partial rotate-half RoPE over first 32 dims, causal, out projection.

Sharding: tensor-parallel over heads. Core h owns Q heads [4h, 4h+4) and
KV head h (one GQA group), i.e. wqkv column shard (2048, 384) and wo
column shard (2048, 256). Each core computes its 4 heads' attention
output in a transposed layout attnT (256, 2048) = (feature, seq),
AllGathers the 8 shards into the full attnT (2048, 2048), and computes
its 256 output columns: outT = wo_shard^T-ish via PE matmul
(lhsT=wo tile, rhs=attnT tile). Host reassembles.

All device compute in bf16 (f32 PSUM accumulation); inputs are cast and
pre-transposed on the host so the device never transposes x.
"""

import os
import sys
import types
import numpy as np
import ml_dtypes

BF16 = ml_dtypes.bfloat16

S = 2048
DIM = 2048
N_HEAD = 32
N_KV = 8
HEAD_DIM = 64
ROPE = 32
N_CORES = 8
QH_PER_CORE = N_HEAD // N_KV          # 4 query heads per core
QCOLS = QH_PER_CORE * HEAD_DIM        # 256 q columns per core
WSH = QCOLS + 2 * HEAD_DIM            # 384 wqkv shard columns
CW = 1024                             # s_q chunk width for attention
NCH = S // CW                         # chunks
TT_P = 128                            # t tile partition size
KT = DIM // 128                       # 16 k tiles for dense matmuls

_COMPILED = None


def _install_ntff_hook():
    """Shim antenv.axon_hooks so bass_utils can NTFF-profile under axon."""
    try:
        import antenv
        if 'antenv.axon_hooks' in sys.modules:
            return
        mod = types.ModuleType('antenv.axon_hooks')
        mod._hook = None

        def set_axon_ntff_profile_hook(h):
            mod._hook = h

        def get_axon_ntff_profile_hook():
            return mod._hook

        mod.set_axon_ntff_profile_hook = set_axon_ntff_profile_hook
        mod.get_axon_ntff_profile_hook = get_axon_ntff_profile_hook
        sys.modules['antenv.axon_hooks'] = mod
        antenv.axon_hooks = mod
        try:
            from trn_agent_boot.trn_boot import _ntff_profile_via_ctypes
            hook = _ntff_profile_via_ctypes('/opt/axon/libaxon_pjrt.so')
            if hook is not None:
                mod._hook = hook
        except Exception:
            pass
    except Exception:
        pass


def build_kernel():
    import concourse.bass as bass
    import concourse.mybir as mybir
    import concourse.tile as tile
    from concourse import bacc
    from concourse.masks import make_identity

    bf = mybir.dt.bfloat16
    f32 = mybir.dt.float32
    MUL = mybir.AluOpType.mult
    ADD = mybir.AluOpType.add
    EXP = mybir.ActivationFunctionType.Exp

    nc = bacc.Bacc('TRN2', target_bir_lowering=False, debug=False,
                   num_devices=N_CORES)

    xT = nc.dram_tensor('xT', [DIM, S], bf, kind='ExternalInput')
    wqkv = nc.dram_tensor('wqkv', [DIM, WSH], bf, kind='ExternalInput')
    wo = nc.dram_tensor('wo', [DIM, QCOLS], bf, kind='ExternalInput')
    cosf = nc.dram_tensor('cosf', [128, S], bf, kind='ExternalInput')
    sinf = nc.dram_tensor('sinf', [128, S], bf, kind='ExternalInput')
    maskd = nc.dram_tensor('maskd', [128, 128], bf, kind='ExternalInput')
    out_ext = nc.dram_tensor('out', [QCOLS, S], f32, kind='ExternalOutput')

    with tile.TileContext(nc) as tc:
        with (
            tc.tile_pool(name='const', bufs=1) as const_pool,
            tc.tile_pool(name='persist', bufs=1) as persist,
            tc.tile_pool(name='dram', bufs=1, space='DRAM') as dram_pool,
            tc.tile_pool(name='dram_rec', bufs=2, space='DRAM') as dram_rec,
        ):
            # ---- constants ----
            cos_sb = const_pool.tile([128, S], bf)
            sin_sb = const_pool.tile([128, S], bf)
            mask_sb = const_pool.tile([128, 128], bf)
            ident = const_pool.tile([128, 128], bf)
            nc.sync.dma_start(cos_sb[:], cosf[:])
            nc.sync.dma_start(sin_sb[:], sinf[:])
            nc.sync.dma_start(mask_sb[:], maskd[:])
            make_identity(nc, ident[:])

            w_sb = persist.tile([128, KT, WSH], bf)
            nc.sync.dma_start(
                w_sb[:], wqkv[:].rearrange('(o p) n -> p o n', p=128))
            wo_sb = persist.tile([128, KT, QCOLS], bf)
            nc.sync.dma_start(
                wo_sb[:], wo[:].rearrange('(o p) n -> p o n', p=128))

            # ---- qkvT = (x @ w_shard)^T in (feature, seq) layout ----
            # m tile 0 -> q heads 0,1 ; 1 -> q heads 2,3 ; 2 -> [k | v]
            qkvT = [persist.tile([128, S], bf, name=f'qkvT{m}')
                    for m in range(3)]
            with (
                tc.tile_pool(name='xt_pool', bufs=1) as xt_pool,
                tc.tile_pool(name='qkv_psum', bufs=2, space='PSUM') as qp,
            ):
                xt_sb = xt_pool.tile([128, KT, S], bf)
                for kt in range(KT):
                    nc.sync.dma_start(
                        xt_sb[:, kt, :], xT[kt * 128:(kt + 1) * 128, :])
                for m in range(3):
                    for sc in range(4):
                        ps = qp.tile([128, 512], f32, tag='qkvps')
                        for kt in range(KT):
                            nc.tensor.matmul(
                                ps[:],
                                lhsT=w_sb[:, kt, m * 128:(m + 1) * 128],
                                rhs=xt_sb[:, kt, sc * 512:(sc + 1) * 512],
                                start=(kt == 0), stop=(kt == KT - 1))
                        nc.vector.tensor_copy(
                            qkvT[m][:, sc * 512:(sc + 1) * 512], ps[:])

            # ---- RoPE on q (both tiles) and k (rows 0:64 of kv tile) ----
            with tc.tile_pool(name='rope', bufs=1) as rope_pool:
                for m in range(2):
                    qt = qkvT[m]
                    rot = rope_pool.tile([128, S], bf, name=f'rot{m}')
                    nc.vector.memset(rot[:], 0.0)
                    for b in (0, 64):
                        nc.sync.dma_start(rot[b:b + 16, :],
                                          qt[b + 16:b + 32, :])
                        nc.sync.dma_start(rot[b + 16:b + 32, :],
                                          qt[b:b + 16, :])
                    nc.vector.tensor_tensor(rot[:], rot[:], sin_sb[:], MUL)
                    nc.vector.tensor_tensor(qt[:], qt[:], cos_sb[:], MUL)
                    nc.vector.tensor_tensor(qt[:], qt[:], rot[:], ADD)
                kvt = qkvT[2]
                rotk = rope_pool.tile([64, S], bf)
                nc.vector.memset(rotk[:], 0.0)
                nc.sync.dma_start(rotk[0:16, :], kvt[16:32, :])
                nc.sync.dma_start(rotk[16:32, :], kvt[0:16, :])
                nc.vector.tensor_tensor(rotk[:], rotk[:], sin_sb[0:64, :], MUL)
                nc.vector.tensor_tensor(kvt[0:64, :], kvt[0:64, :],
                                        cos_sb[0:64, :], MUL)
                nc.vector.tensor_tensor(kvt[0:64, :], kvt[0:64, :],
                                        rotk[:], ADD)

            # ---- transpose v rows (kv[64:128]) into v_aug (t, d | 1) ----
            v_aug = persist.tile([128, S // 128, HEAD_DIM + 1], bf)
            nc.vector.memset(v_aug[:], 1.0)
            with tc.tile_pool(name='tr_psum', bufs=2, space='PSUM') as trp:
                for i in range(S // 128):
                    pt = trp.tile([128, HEAD_DIM], bf, tag='trps')
                    nc.tensor.transpose(
                        pt[:], qkvT[2][64:128, i * 128:(i + 1) * 128],
                        ident[64:128, 64:128])
                    nc.vector.tensor_copy(v_aug[:, i, 0:HEAD_DIM], pt[:])

            # k duplicated to both partition halves so scores lhsT can
            # match each q head's base partition (matmul requires equal
            # base partitions for lhsT and rhs)
            kk = persist.tile([128, S], bf)
            nc.sync.dma_start(kk[0:64, :], qkvT[2][0:64, :])
            nc.sync.dma_start(kk[64:128, :], qkvT[2][0:64, :])

            # ---- attention + chunked AllGather + out projection ----
            # Software-pipelined per chunk: iteration h interleaves head
            # h's scores/exp with head h-1's PV at t-tile granularity so
            # the in-order PE queue always has dense work; normalization
            # chains (DMA-latency-bound) are emitted where the DVE queue
            # is otherwise idle; AG is split per 512-col half and wo
            # groups of chunk c-1 are interleaved into chunk c's stream.
            with (
                tc.tile_pool(name='sc_psum', bufs=1, space='PSUM') as scp,
                tc.tile_pool(name='pv_psum', bufs=2, space='PSUM') as pvp,
                tc.tile_pool(name='wo_psum', bufs=2, space='PSUM') as wop,
                tc.tile_pool(name='probs', bufs=20) as probs_pool,
                tc.tile_pool(name='smax', bufs=2) as smax_pool,
                tc.tile_pool(name='attn_sb', bufs=3) as attn_pool,
                tc.tile_pool(name='ag_sb', bufs=36) as agp,
                tc.tile_pool(name='out_sb', bufs=2) as outp,
                tc.tile_pool(name='ag_dram', bufs=2, space='DRAM') as agd,
            ):
                def a_step(c, h, tt, probs_list):
                    """scores + exp for (chunk c, head h, t-tile tt)."""
                    q_tile = qkvT[h // 2]
                    qp0 = (h % 2) * 64
                    start = max(0, 128 * tt - CW * c)
                    ps = scp.tile([128, CW], f32, tag='scps')
                    p0 = start
                    while p0 < CW:
                        p1 = min(p0 + 512 - (p0 % 512), CW)
                        nc.tensor.matmul(
                            ps[:, p0:p1],
                            lhsT=kk[qp0:qp0 + 64, tt * 128:(tt + 1) * 128],
                            rhs=q_tile[qp0:qp0 + 64,
                                       c * CW + p0:c * CW + p1],
                            start=True, stop=True)
                        p0 = p1
                    probs = probs_pool.tile([128, CW], bf, tag='pb')
                    if start % 512 > 0:
                        z0 = (start // 512) * 512
                        nc.vector.memset(probs[:, z0:start], 0.0)
                    nc.scalar.activation(
                        probs[:, start:CW], ps[:, start:CW], EXP, scale=0.125)
                    if 128 * tt >= CW * c:  # diagonal tile
                        nc.vector.tensor_tensor(
                            probs[:, start:start + 128],
                            probs[:, start:start + 128], mask_sb[:], MUL)
                    probs_list.append(probs)

                def b_step(c, tt, n_tt, pv, probs):
                    """PV accumulate for one t-tile."""
                    start = max(0, 128 * tt - CW * c)
                    for half in range(CW // 512):
                        if start < 512 * (half + 1):
                            last_tt = min(
                                n_tt - 1,
                                (CW * c + 512 * (half + 1)) // 128 - 1)
                            nc.tensor.matmul(
                                pv[:, half * 512:(half + 1) * 512],
                                lhsT=v_aug[:, tt, :],
                                rhs=probs[:, half * 512:(half + 1) * 512],
                                start=(tt == 0), stop=(tt == last_tt),
                                skip_group_check=True)

                def norm_chain(c, h, pv, ag_halves):
                    """softmax divide via ones-column sums + store shard."""
                    attn_un = attn_pool.tile([64, CW], bf, tag='attnu')
                    nc.vector.tensor_copy(attn_un[:], pv[0:HEAD_DIM, :])
                    den = smax_pool.tile([HEAD_DIM + 1, CW], f32, tag='den')
                    nc.vector.tensor_copy(
                        den[HEAD_DIM:HEAD_DIM + 1, :],
                        pv[HEAD_DIM:HEAD_DIM + 1, :])
                    den_p = smax_pool.tile([128, CW // 128], f32, tag='denp')
                    nc.sync.dma_start(
                        den_p[:], den[HEAD_DIM:HEAD_DIM + 1, :])
                    rec_p = smax_pool.tile([128, CW // 128], f32, tag='recp')
                    nc.vector.reciprocal(rec_p[:], den_p[:])
                    rec_dram = dram_rec.tile([1, CW], f32, tag='recd')
                    nc.sync.dma_start(rec_dram[:], rec_p[:])
                    rec_bc = smax_pool.tile([64, CW], f32, tag='recbc')
                    nc.sync.dma_start(
                        rec_bc[:], rec_dram[:].to_broadcast((64, CW)))
                    attn_n = attn_pool.tile([64, CW], bf, tag='attnn')
                    nc.vector.tensor_tensor(
                        attn_n[:], attn_un[:], rec_bc[:], MUL)
                    for half in range(CW // 512):
                        nc.sync.dma_start(
                            ag_halves[half][h * 64:(h + 1) * 64, :],
                            attn_n[:, half * 512:(half + 1) * 512])

                def wo_group(c, half, et, ag_tiles):
                    pso = wop.tile([128, 512], f32, tag='wops')
                    for ft in range(KT):
                        nc.tensor.matmul(
                            pso[:],
                            lhsT=wo_sb[:, ft, et * 128:(et + 1) * 128],
                            rhs=ag_tiles[ft][:, 0:512],
                            start=(ft == 0), stop=(ft == KT - 1))
                    osb = outp.tile([128, 512], f32, tag='osb')
                    nc.vector.tensor_copy(osb[:], pso[:])
                    nc.sync.dma_start(
                        out_ext[et * 128:(et + 1) * 128,
                                c * CW + half * 512:
                                c * CW + (half + 1) * 512], osb[:])

                def load_ag_tiles(ag_out):
                    # gpsimd queue: these waits must not block the sync
                    # queue's normalization DMA chains
                    tiles = []
                    for ft in range(KT):
                        agt = agp.tile([128, 512], bf, tag='agt')
                        nc.gpsimd.dma_start(
                            agt[:], ag_out[ft * 128:(ft + 1) * 128, :])
                        tiles.append(agt)
                    return tiles

                # wo work items from the previous chunk, emitted one per
                # pipeline slot: (c, half, et, ag_out or ag_tiles)
                pending_wo = []

                def emit_one_wo():
                    if pending_wo:
                        c_, half_, et_, tiles_ = pending_wo.pop(0)
                        wo_group(c_, half_, et_, tiles_)

                for c in range(NCH):
                    n_tt = (CW // 128) * (c + 1)
                    probs_by_head = {}
                    pv_by_head = {}
                    ag_halves = [agd.tile([QCOLS, 512], bf, tag=f'agin{half}',
                                          name=f'agin{half}')
                                 for half in range(CW // 512)]
                    for h in range(QH_PER_CORE + 1):
                        if h <= QH_PER_CORE - 1:
                            probs_by_head[h] = []
                        if h >= 1:
                            pv_by_head[h - 1] = pvp.tile(
                                [HEAD_DIM + 1, CW], f32, tag='pv',
                                name=f'pv_{h - 1}')
                        for tt in range(n_tt):
                            if h <= QH_PER_CORE - 1:
                                a_step(c, h, tt, probs_by_head[h])
                            if h >= 1:
                                b_step(c, tt, n_tt, pv_by_head[h - 1],
                                       probs_by_head[h - 1][tt])
                        if h >= 1:
                            norm_chain(c, h - 1, pv_by_head[h - 1],
                                       ag_halves)
                            emit_one_wo()
                    # chunk done: AllGather each half, queue wo groups
                    for half in range(CW // 512):
                        ag_out = agd.tile([N_HEAD * HEAD_DIM, 512], bf,
                                          addr_space='Shared',
                                          tag=f'agout{half}')
                        nc.gpsimd.collective_compute(
                            'AllGather', mybir.AluOpType.bypass,
                            replica_groups=[list(range(N_CORES))],
                            ins=[ag_halves[half][:].opt()],
                            outs=[ag_out[:].opt()])
                        tiles = load_ag_tiles(ag_out)
                        for et in range(2):
                            pending_wo.append((c, half, et, tiles))
                while pending_wo:
                    emit_one_wo()

    nc.compile()
    return nc


def _prepare_in_maps(x, cos, sin, wqkv, wo):
    x2 = np.ascontiguousarray(np.asarray(x, dtype=np.float32).reshape(S, DIM))
    xT = np.ascontiguousarray(x2.T).astype(BF16)
    cos2 = np.asarray(cos, dtype=np.float32).reshape(S, ROPE)
    sin2 = np.asarray(sin, dtype=np.float32).reshape(S, ROPE)
    cosT = np.ascontiguousarray(cos2.T)  # (32, S)
    sinT = np.ascontiguousarray(sin2.T)

    # cos_full: blocks of 64 rows: [cos(32) | ones(32)] twice
    cos_full = np.ones((128, S), dtype=np.float32)
    sin_full = np.zeros((128, S), dtype=np.float32)
    for b in (0, 64):
        cos_full[b:b + 32] = cosT
        sin_full[b:b + 16] = -sinT[0:16]
        sin_full[b + 16:b + 32] = sinT[16:32]
    cos_full = cos_full.astype(BF16)
    sin_full = sin_full.astype(BF16)

    # lower-triangle-inclusive mask for diagonal 128x128 blocks:
    # keep (p, f) iff f >= p
    mask = (np.arange(128)[None, :] >= np.arange(128)[:, None])
    mask = mask.astype(BF16)

    wq = np.asarray(wqkv, dtype=np.float32)
    wov = np.asarray(wo, dtype=np.float32)
    in_maps = []
    for h in range(N_CORES):
        w_shard = np.concatenate([
            wq[:, h * QCOLS:(h + 1) * QCOLS],
            wq[:, DIM + h * HEAD_DIM:DIM + (h + 1) * HEAD_DIM],
            wq[:, DIM + N_KV * HEAD_DIM + h * HEAD_DIM:
               DIM + N_KV * HEAD_DIM + (h + 1) * HEAD_DIM],
        ], axis=1).astype(BF16)
        wo_shard = np.ascontiguousarray(
            wov[:, h * QCOLS:(h + 1) * QCOLS]).astype(BF16)
        in_maps.append({
            'xT': xT,
            'wqkv': np.ascontiguousarray(w_shard),
            'wo': wo_shard,
            'cosf': cos_full,
            'sinf': sin_full,
            'maskd': np.ascontiguousarray(mask),
        })
    return in_maps


def kernel(x, cos, sin, wqkv, wo):
    global _COMPILED
    from concourse.bass_utils import run_bass_kernel_spmd

    _install_ntff_hook()
    if _COMPILED is None:
        _COMPILED = build_kernel()
    nc = _COMPILED

    in_maps = _prepare_in_maps(x, cos, sin, wqkv, wo)
    trace = bool(os.environ.get('BASS_KERNEL_TRACE'))
    tmpdir = os.environ.get('BASS_KERNEL_TRACE_DIR') or None
    res = run_bass_kernel_spmd(nc, in_maps, list(range(N_CORES)),
                               trace=trace, tmpdir=tmpdir)
    if trace:
        kernel.last_exec_time_ns = res.exec_time_ns

    out = np.empty((S, DIM), dtype=np.float32)
    for h in range(N_CORES):
        out[:, h * QCOLS:(h + 1) * QCOLS] = \
            np.asarray(res.results[h]['out'], dtype=np.float32).T
    return out.reshape(1, S, DIM)


kernel.last_exec_time_ns = None
